# revision 26
# baseline (speedup 1.0000x reference)
"""Conditional contrastive loss on 8 TRN2 NeuronCores (Bass/Tile).

Strategy (data-parallel over rows, per sharding hint):
  - Each core owns 512 rows (of 4096) of inst_embed ("x") and proxy ("p").
  - The host row-normalizes x and p in fp32 and ships fp8(e4m3) operands
    in the exact on-chip layout: the full normalized xn^T (matmul rhs,
    k-chunked, four column-quarter tiles), the core's own xn/pn columns
    (matmul lhsT, both matrices in one 4KB-row tensor), and the core's
    pre-gathered positive-selection mask rows negative_mask[labels]
    (fp8; 0/1 exact). This removes the entire on-device normalization
    pipeline, and every big DMA moves fully-contiguous 4KB rows (the
    DMA fabric moves fixed ~158ns 4KB packet-slots across 16 engines).
  - Similarity rows sim[i, j] for the core's i-block: fp8 DoubleRow
    matmuls (2 contraction rows per PE cell -> K=256 per instruction)
    accumulated in PSUM, 2048 columns per PSUM group, double-buffered.
    A zero-matmul warmup stream keeps the PE HAM-warm through the DMA
    preamble, and the first group is split into two half-groups so the
    exp/mask pipeline starts as soon as the first quarter lands.
  - exp((sim-margin)/T) on the scalar engine straight out of PSUM with
    accum_out = per-group row sums -> denominator; z to SBUF in bf16.
  - numerator = scalar_tensor_tensor(z * mask) on DVE with accum_out
    (the DVE 1x fused op is the steady-state gate; TT/TS alternatives
    measure slower because fp8/accum variants have no 2x uops).
  - Device emits raw per-group (den, num) row sums (p-major [128, 36]
    f32); the host does the final group-sum/log/mean across cores.
"""
import numpy as np
import ml_dtypes

import concourse.bacc as bacc
import concourse.tile as tile
from concourse import mybir, bass_utils

N_FULL = 4096
D = 512
N_CORES = 8
RP = N_FULL // N_CORES  # rows per core = 512
P = 128                 # SBUF partitions
KC = D // P             # 128-row contraction chunks = 4
JT = 512                # columns per PSUM bank
JG = 2048               # columns per PSUM group (4 banks)
JQ = 1024               # columns per rhs quarter-tile
NG = N_FULL // JG       # groups per (i-tile, matrix) = 2
IT = RP // P            # i-tiles per core = 4

F32 = mybir.dt.float32
BF16 = mybir.dt.bfloat16
F8 = mybir.dt.float8e4
AF = mybir.ActivationFunctionType
ALU = mybir.AluOpType
DR = mybir.MatmulPerfMode.DoubleRow

_CACHE = {}


def _build(inv_t: float, bias_den: float):
    nc = bacc.Bacc("TRN2", target_bir_lowering=False, debug=False,
                   num_devices=N_CORES)

    # xdr layout: [g, h, k, n] quarters; 4KB contiguous rows per quarter
    xdr = nc.dram_tensor("xdr", [P, NG * 2 * KC * JQ], F8,
                         kind="ExternalInput")
    # wcc[p, :KC*RP] = proxy weights [k*RP+m]; [KC*RP:] = inst weights
    wcc = nc.dram_tensor("wcc", [P, 2 * KC * RP], F8, kind="ExternalInput")
    mk = nc.dram_tensor("mk", [RP, N_FULL], F8, kind="ExternalInput")
    # p-major output; host de-interleaves [p, it*8+c] -> [it*128+p, c]
    out = nc.dram_tensor("out", [P, 36], F32, kind="ExternalOutput")

    with tile.TileContext(nc) as tc:
        with (
            tc.tile_pool(name="xpool", bufs=1) as xpool,
            tc.tile_pool(name="lhs", bufs=1) as lhs,
            tc.tile_pool(name="maskp", bufs=1) as maskp,
            tc.tile_pool(name="zpool", bufs=6) as zpool,
            tc.tile_pool(name="zopool", bufs=2) as zopool,
            tc.tile_pool(name="small", bufs=1) as small,
            tc.tile_pool(name="ps", bufs=2, space="PSUM") as pspool,
        ):
            # ---- constants (no DMA deps; emitted first) ----
            zeros_w = small.tile([P, P], BF16, name="zeros_w")
            nc.vector.memset(zeros_w[:], 0.0)
            zeros_r = small.tile([P, JT], BF16, name="zeros_r")
            nc.vector.memset(zeros_r[:], 0.0)
            dummy = small.tile([P, 1], F32, name="dummy")
            nc.vector.memset(dummy[:], 0.0)
            # trigger the ~2.7us exp table-set load during the DMA preamble
            nc.scalar.activation(dummy[:], dummy[:], AF.Exp)

            # ---- loads: one ring (engines are shared), by first use ----
            wc = lhs.tile([P, 2 * KC * RP], F8, name="wc")
            xh = [xpool.tile([P, KC * JQ], F8, name=f"xh{q}")
                  for q in range(2 * NG)]
            mask_t = [maskp.tile([P, N_FULL], F8, name=f"mask{it}")
                      for it in range(IT)]
            WQ = KC * JQ
            nc.sync.dma_start(wc[:], wcc.ap())
            nc.sync.dma_start(xh[0][:], xdr.ap()[:, 0:WQ])
            nc.sync.dma_start(xh[1][:], xdr.ap()[:, WQ:2 * WQ])
            # mask0 split: first half covers the split first group's STTs
            nc.sync.dma_start(mask_t[0][:, 0:JG], mk.ap()[0:P, 0:JG])
            nc.sync.dma_start(xh[2][:], xdr.ap()[:, 2 * WQ:3 * WQ])
            nc.sync.dma_start(xh[3][:], xdr.ap()[:, 3 * WQ:4 * WQ])
            nc.sync.dma_start(mask_t[0][:, JG:N_FULL], mk.ap()[0:P, JG:N_FULL])
            for it in range(1, IT):
                nc.sync.dma_start(mask_t[it][:],
                                  mk.ap()[it * P:(it + 1) * P, :])

            # 3D views for DoubleRow slicing: [P, k-chunk, cols]
            xh3 = [t[:].rearrange("p (k n) -> p k n", k=KC) for t in xh]
            wp3 = wc[:, 0:KC * RP].rearrange("p (k m) -> p k m", k=KC)
            wx3 = wc[:, KC * RP:].rearrange("p (k m) -> p k m", k=KC)

            # acc columns: it*8 + mat*4 + [0/1]=den(g0,g1), [2/3]=num;
            # cols 32/34 hold the split first group's second-half den/num.
            acc = small.tile([P, 36], F32, name="acc")

            def tile_work(it, mat, g, h=None, first=False):
                """One PSUM group (or a half-group when h is given)."""
                i0 = it * P
                w3 = wp3 if mat == 0 else wx3
                cols = JG if h is None else JQ
                ps = pspool.tile([P, cols], F32,
                                 name=f"ps_{it}_{mat}_{g}_{h}", tag="ps")
                if first:
                    # HAM warm-up: zero matmuls keep the PE busy while
                    # input DMAs stream; HAM reaches 8/8 mid-warmup and
                    # real matmuls queue in with no idle gap.
                    for w in range(11):
                        nc.tensor.matmul(
                            ps[:, 0:JT], zeros_w[:], zeros_r[:],
                            start=(w == 0), stop=(w == 10),
                        )
                jls = range(4) if h is None else range(2 * h, 2 * h + 2)
                for b in range(2):  # DoubleRow K-blocks (256 each)
                    ksl = slice(2 * b, 2 * b + 2)
                    for i, jl in enumerate(jls):
                        q = g * 2 + jl // 2
                        c0 = (jl % 2) * JT
                        nc.tensor.matmul(
                            ps[:, i * JT:(i + 1) * JT],
                            w3[:, ksl, i0:i0 + P],
                            xh3[q][:, ksl, c0:c0 + JT],
                            start=(b == 0), stop=(b == 1),
                            perf_mode=DR,
                        )
                z = zpool.tile([P, cols], BF16,
                               name=f"z_{it}_{mat}_{g}_{h}", tag="z")
                zo = zopool.tile([P, cols], BF16,
                                 name=f"zo_{it}_{mat}_{g}_{h}", tag="zo")
                cd = it * 8 + mat * 4 + g
                cn = cd + 2
                if h == 1:
                    cd, cn = 32, 34
                nc.scalar.activation(
                    z[:], ps[:], AF.Exp,
                    bias=bias_den, scale=inv_t,
                    accum_out=acc[:, cd:cd + 1],
                )
                m0 = g * JG + (0 if h is None else h * JQ)
                nc.vector.scalar_tensor_tensor(
                    out=zo[:], in0=z[:], scalar=1.0,
                    in1=mask_t[it][:, m0:m0 + cols],
                    op0=ALU.mult, op1=ALU.mult,
                    accum_out=acc[:, cn:cn + 1],
                )

            # ---- main loop; first group split into two half-groups ----
            tile_work(0, 0, 0, h=0, first=True)
            tile_work(0, 0, 0, h=1)
            tile_work(0, 1, 0)
            tile_work(0, 0, 1)
            tile_work(0, 1, 1)
            for it in range(1, IT):
                for g in range(NG):
                    for mat in range(2):
                        tile_work(it, mat, g)

            nc.sync.dma_start(out.ap()[:], acc[:])

    nc.compile()
    return nc


def _chunked(aT):
    """[D, n] -> [128, KC * n] with free layout [k-chunk, col]."""
    return np.ascontiguousarray(
        aT.reshape(KC, P, -1).transpose(1, 0, 2).reshape(P, -1))


def make_in_maps(x, p, nmf, lab):
    eps = 1e-8
    xn = x / np.maximum(np.linalg.norm(x, axis=-1, keepdims=True), eps)
    pn = p / np.maximum(np.linalg.norm(p, axis=-1, keepdims=True), eps)
    xnT = xn.T.astype(ml_dtypes.float8_e4m3)
    pnT = pn.T.astype(ml_dtypes.float8_e4m3)
    # xdr free layout: [g, h, k, n-in-quarter]
    xdr = np.ascontiguousarray(
        xnT.reshape(KC, P, NG, 2, JQ).transpose(1, 2, 3, 0, 4).reshape(P, -1))
    in_maps = []
    for c in range(N_CORES):
        rows = slice(c * RP, (c + 1) * RP)
        in_maps.append({
            "xdr": xdr,
            "wcc": np.concatenate(
                [_chunked(pnT[:, rows]), _chunked(xnT[:, rows])], axis=1),
            "mk": nmf[lab[rows]].astype(ml_dtypes.float8_e4m3),
        })
    return in_maps


def kernel(inst_embed, proxy, negative_mask, labels, temperature, margin):
    t = float(np.asarray(temperature))
    m = float(np.asarray(margin))
    inv_t = 1.0 / t
    bias_den = -m / t

    key = (t, m)
    if key not in _CACHE:
        _CACHE[key] = _build(inv_t, bias_den)
    nc = _CACHE[key]

    x = np.asarray(inst_embed, dtype=np.float32)
    p = np.asarray(proxy, dtype=np.float32)
    nmf = np.asarray(negative_mask, dtype=np.float32)
    lab = np.asarray(labels).astype(np.int64)

    in_maps = make_in_maps(x, p, nmf, lab)

    res = bass_utils.run_bass_kernel_spmd(nc, in_maps,
                                          core_ids=list(range(N_CORES)))
    # out is p-major [128, it*8+c] (+ split-group extras in cols 32/34)
    den_p = np.zeros(N_FULL)
    num_p = np.zeros(N_FULL)
    den_i = np.zeros(N_FULL)
    num_i = np.zeros(N_FULL)
    for c in range(N_CORES):
        o = np.asarray(res.results[c]["out"], dtype=np.float64)
        for it in range(IT):
            rows = slice(c * RP + it * P, c * RP + (it + 1) * P)
            b = it * 8
            dp = o[:, b + 0] + o[:, b + 1]
            npv = o[:, b + 2] + o[:, b + 3]
            if it == 0:
                dp = dp + o[:, 32]
                npv = npv + o[:, 34]
            den_p[rows] = dp
            num_p[rows] = npv
            den_i[rows] = o[:, b + 4] + o[:, b + 5]
            num_i[rows] = o[:, b + 6] + o[:, b + 7]
    loss = (-2.0 * np.log(t)
            + (np.log(den_p) - np.log(num_p)).mean()
            + (np.log(den_i) - np.log(num_i)).mean())
    return np.float32(loss)
